# revision 1
# baseline (speedup 1.0000x reference)
"""IDW k-NN flow interpolation (cdist -> top-8 -> inverse-distance-weighted
gather) on 8 Trainium2 NeuronCores.

Sharding: queries split 8 ways (4096/core); ref points/flows replicated.

Per-core Bass kernel, per 128-query tile:
  1. PE matmul (K=4): score[i,j] = 2*q_i.r_j - |r_j|^2  (= -dist^2 + |q_i|^2,
     so per-row ordering by score == ordering by -dist^2).
  2. ACT evicts PSUM chunks into an SBUF [128, M] score tile.
  3. DVE max8 -> top-8 scores per query; find_index8 -> their column indices.
  4. Small-tile weight math: d2 = relu(q_sq - score), w = 1/(d2+1e-8),
     normalized.
  5. Indirect DMA gathers the 8 flow rows per query; weighted sum -> out.
"""

import os
import sys

import numpy as np

for _p in ("/opt/trn_rl_repo", "/root/.axon_site/_ro/trn_rl_repo"):
    if os.path.isdir(_p) and _p not in sys.path:
        sys.path.append(_p)

from concourse import bass, mybir  # noqa: E402
from concourse import tile  # noqa: E402
from concourse.bass_utils import run_bass_kernel_spmd  # noqa: E402

N_FULL = 32768
M_FULL = 16384
D = 3
K = 8
N_CORES = 8
P = 128
CH = 512  # psum chunk (one bank of fp32)


def build_module(n_loc=N_FULL // N_CORES, m=M_FULL, stage=3, split=True):
    """Build the single-core Bass module (queries [n_loc], refs [m]).

    stage: debug knob — 1 = matmul+evict only, 2 = +top8 (no gather),
    3 = full kernel.
    """
    nt = n_loc // P
    nch = m // CH
    assert n_loc % P == 0 and m % CH == 0

    nc = bass.Bass("TRN2", debug=False)

    G = 4  # PE 32-partition row groups; refs split G ways along m
    mg = m // G
    assert mg % CH == 0
    # comb rows (per group g): 4 rows x (mg ref columns ++ n_loc query
    # columns). Ref cols: [r_sq, 2rx, 2ry, 2rz]; query cols: [-1, qx, qy, qz].
    comb_d = nc.dram_tensor(
        "comb", [4 * G, mg + n_loc], mybir.dt.float32, kind="ExternalInput"
    )
    q_d = nc.dram_tensor("q", [n_loc, D], mybir.dt.float32, kind="ExternalInput")
    flow_d = nc.dram_tensor("flow", [m, D], mybir.dt.float32, kind="ExternalInput")
    out_d = nc.dram_tensor("out", [n_loc, D], mybir.dt.float32, kind="ExternalOutput")

    AF = mybir.ActivationFunctionType
    AX = mybir.AxisListType
    OP = mybir.AluOpType

    with tile.TileContext(nc) as tc:
        with (
            tc.tile_pool(name="const", bufs=1) as cpool,
            tc.tile_pool(name="score", bufs=2) as spool,
            tc.tile_pool(name="psum", bufs=4, space="PSUM") as ppool,
            tc.tile_pool(name="small", bufs=16) as mpool,
        ):
            # All per-core inputs are small: load once up-front. Group g's
            # refs+queries live on partitions 32g..32g+3, loaded by ONE DMA
            # so every matmul depends on a single DMA-lane semaphore (the
            # fp32 LDWEIGHTS micro-op has only 2 sem-wait slots).
            comb = cpool.tile([P, mg + n_loc], mybir.dt.float32)
            for g in range(G):
                nc.gpsimd.dma_start(
                    out=comb[32 * g : 32 * g + 4, :],
                    in_=comb_d[4 * g : 4 * g + 4, :],
                )
            # natural-layout queries: [128, nt, 3] (query t*128+p at [p, t, :])
            q_all = cpool.tile([P, nt, D], mybir.dt.float32)
            nc.gpsimd.dma_start(
                out=q_all[:, :, :],
                in_=q_d[:, :].rearrange("(t p) d -> p t d", p=P),
            )
            out_all = cpool.tile([P, nt, D], mybir.dt.float32)
            ones8 = cpool.tile([P, K], mybir.dt.float32)
            nc.gpsimd.memset(ones8[:, :], 1.0)

            ncg = mg // CH  # chunks per group
            for t in range(nt):
                qs = slice(t * P, (t + 1) * P)

                # q_sq [128,1] — adds on GPSIMD, DVE stays clear
                q3s = mpool.tile([P, D], mybir.dt.float32, tag="q3s")
                nc.scalar.activation(q3s[:, :], q_all[:, t, :], AF.Square)
                qsq = mpool.tile([P, 2], mybir.dt.float32, tag="qsq")
                nc.gpsimd.tensor_tensor(
                    qsq[:, 1:2], q3s[:, 0:1], q3s[:, 1:2], op=OP.add
                )
                nc.gpsimd.tensor_tensor(
                    qsq[:, 0:1], qsq[:, 1:2], q3s[:, 2:3], op=OP.add
                )

                # score tile: PE matmul chunks -> PSUM -> ACT evict -> SBUF
                score = spool.tile([P, m], mybir.dt.float32, tag="score")
                for c in range(nch):
                    g, lc = c // ncg, c % ncg
                    pb = 32 * g
                    lcs = slice(lc * CH, (lc + 1) * CH)
                    ps = ppool.tile([P, CH], mybir.dt.float32, tag="ps")
                    nc.tensor.matmul(
                        ps[:, :],
                        lhsT=comb[pb : pb + 4, mg + t * P : mg + (t + 1) * P],
                        rhs=comb[pb : pb + 4, lcs],
                        start=True, stop=True,
                        tile_position=(pb, 0),
                    )
                    nc.scalar.copy(score[:, c * CH : (c + 1) * CH], ps[:, :])

                if stage < 2:
                    nc.vector.tensor_copy(out_all[:, t, :], score[:, 0:3])
                    continue

                # top-8 scores (descending) and their indices. For tile 0
                # only, run max8 per quarter-row and merge, so the scan
                # starts as soon as the first quarter is evicted instead of
                # waiting out the whole first tile (cuts the startup fill).
                vals = mpool.tile([P, K], mybir.dt.float32, tag="vals")
                if t == 0 and m % (4 * CH) == 0:
                    mq = m // 4
                    vq = mpool.tile([P, 4, K], mybir.dt.float32, tag="vq")
                    for i in range(4):
                        nc.vector.max(
                            vq[:, i, :], score[:, i * mq : (i + 1) * mq]
                        )
                    nc.vector.max(vals[:, :], vq[:, :, :])
                else:
                    nc.vector.max(vals[:, :], score[:, :])
                idxs = mpool.tile([P, K], mybir.dt.uint32, tag="idxs")
                nc.vector.max_index(idxs[:, :], vals[:, :], score[:, :])

                # weights: d2 = relu(q_sq - score) ; w = 1/(d2 + 1e-8) ;
                # normalize. Small ops go to GPSIMD/ACT — DVE is the
                # kernel bottleneck (max8 + find_index8), keep it clear.
                nd2 = mpool.tile([P, K], mybir.dt.float32, tag="nd2")
                nc.gpsimd.tensor_scalar(
                    nd2[:, :], vals[:, :], qsq[:, :1], None, op0=OP.subtract
                )
                d2 = mpool.tile([P, K], mybir.dt.float32, tag="d2")
                nc.scalar.activation(d2[:, :], nd2[:, :], AF.Relu, scale=-1.0)
                nc.gpsimd.tensor_scalar_add(d2[:, :], d2[:, :], 1e-8)
                w = mpool.tile([P, K], mybir.dt.float32, tag="w")
                nc.vector.reciprocal(w[:, :], d2[:, :])
                # ws = sum_k w (add tree on GPSIMD) ; wr = 1/ws ; wn = w * wr
                wt = mpool.tile([P, 4 + 2 + 2], mybir.dt.float32, tag="wt")
                nc.gpsimd.tensor_tensor(wt[:, 0:4], w[:, 0:4], w[:, 4:8], op=OP.add)
                nc.gpsimd.tensor_tensor(wt[:, 4:6], wt[:, 0:2], wt[:, 2:4], op=OP.add)
                nc.gpsimd.tensor_tensor(wt[:, 6:7], wt[:, 4:5], wt[:, 5:6], op=OP.add)
                nc.vector.reciprocal(wt[:, 7:8], wt[:, 6:7])
                wn = mpool.tile([P, K], mybir.dt.float32, tag="wn")
                nc.gpsimd.tensor_scalar_mul(wn[:, :], w[:, :], wt[:, 7:8])

                # gather the 8 flow rows per query: [128, 8, 3]
                fg = mpool.tile([P, K, D], mybir.dt.float32, tag="fg")
                if stage < 3:
                    nc.vector.memset(fg[:, :, :], 1.0)
                else:
                    # one offset per partition per indirect DMA — the
                    # multi-offset form crashes the exec unit on HW
                    for k in range(K):
                        nc.gpsimd.indirect_dma_start(
                            out=fg[:, k, :],
                            out_offset=None,
                            in_=flow_d[:, :],
                            in_offset=bass.IndirectOffsetOnAxis(
                                ap=idxs[:, k : k + 1], axis=0
                            ),
                        )

                # weighted sum over k: prodT [128, 3, 8] = fg[:, k, c] * wn[:, k]
                prodT = mpool.tile([P, D, K], mybir.dt.float32, tag="prodT")
                for c in range(D):
                    nc.gpsimd.tensor_tensor(
                        out=prodT[:, c, :],
                        in0=fg[:, :, c],
                        in1=wn[:, :],
                        op=OP.mult,
                    )
                pr4 = mpool.tile([P, D, 4], mybir.dt.float32, tag="pr4")
                nc.gpsimd.tensor_tensor(
                    pr4[:, :, :], prodT[:, :, 0:4], prodT[:, :, 4:8], op=OP.add
                )
                nc.gpsimd.tensor_tensor(
                    pr4[:, :, 0:2], pr4[:, :, 0:2], pr4[:, :, 2:4], op=OP.add
                )
                nc.gpsimd.tensor_tensor(
                    out_all[:, t, :, None], pr4[:, :, 0:1], pr4[:, :, 1:2], op=OP.add
                )

            nc.gpsimd.dma_start(
                out=out_d[:, :].rearrange("(t p) d -> p t d", p=P),
                in_=out_all[:, :, :],
            )

    if split:
        _split_waits(nc)
    return nc


_SPLIT_SEQ = [0]


def _split_waits(nc, limit=1):
    """Move excess sem-waits onto preceding same-engine NOPs.

    Several TRN2 ISA structs accept only a small number of sync-wait
    commands and walrus refuses to split them ("Too many sync wait
    commands"). A NOP carrying one wait is always legal, and a wait
    executed earlier on the same engine is strictly more conservative,
    so this preserves correctness.
    """
    import concourse.mybir as mybir  # noqa: PLC0415
    from concourse.tile_rust import add_dep_helper  # noqa: PLC0415

    for fn in nc.m.functions:
        for b in fn.blocks:
            il = b.instructions
            idx = 0
            while idx < len(il):
                inst = il[idx]
                si = inst.sync_info
                if si is not None and len(si.on_wait) > limit:
                    waits = list(si.on_wait)
                    excess, keep = waits[:-limit], waits[-limit:]
                    inst.sync_info = mybir.SyncInfo(
                        on_wait=keep, on_update=list(si.on_update)
                    )
                    # previous same-engine instruction, to pin the nops'
                    # position in that engine's stream
                    prev = None
                    for k in range(idx - 1, -1, -1):
                        if il[k].engine == inst.engine:
                            prev = il[k]
                            break
                    chain = prev
                    for j, w in enumerate(excess):
                        _SPLIT_SEQ[0] += 1
                        nop = mybir.InstNoOp(
                            name=f"waitnop-{_SPLIT_SEQ[0]}", ins=[], outs=[]
                        )
                        nop.engine = inst.engine
                        nop.sync_info = mybir.SyncInfo(on_wait=[w], on_update=[])
                        nc.register_instruction(nop, overwrite=True)
                        if chain is not None:
                            add_dep_helper(nop, chain, True, "waitnop order")
                        chain = nop
                        il.insert(idx + j, nop)
                    add_dep_helper(inst, chain, True, "waitnop order")
                    idx += len(excess)
                idx += 1


def pack_inputs(query_points, ref_points, ref_flow):
    """Host-side input marshalling: shard queries, pack/replicate refs."""
    q = np.ascontiguousarray(np.asarray(query_points, dtype=np.float32))
    r = np.ascontiguousarray(np.asarray(ref_points, dtype=np.float32))
    f = np.ascontiguousarray(np.asarray(ref_flow, dtype=np.float32))
    n, m = q.shape[0], r.shape[0]
    n_loc = n // N_CORES

    G = 4
    mg = m // G

    in_maps = []
    for c in range(N_CORES):
        sl = slice(c * n_loc, (c + 1) * n_loc)
        comb = np.empty((4 * G, mg + n_loc), dtype=np.float32)
        for g in range(G):
            rs = r[g * mg : (g + 1) * mg]
            comb[4 * g, :mg] = (rs * rs).sum(axis=1)
            comb[4 * g + 1 : 4 * g + 4, :mg] = (2.0 * rs).T
            comb[4 * g, mg:] = -1.0
            comb[4 * g + 1 : 4 * g + 4, mg:] = q[sl].T
        in_maps.append({"comb": comb, "q": q[sl], "flow": f})
    return in_maps


_NC_CACHE = {}


def _get_module(n_loc, m):
    """Build + verify-compile the module.

    The Tile scheduler is process-nondeterministic (rust hash seeds) and
    some schedules emit more sem-waits on an instruction than its ISA
    struct allows, which walrus rejects. walrus is deterministic given a
    BIR, so: rebuild until a test-compile passes, then reuse that module
    for the real run (same BIR -> same walrus outcome).
    """
    import tempfile

    from concourse.bass_utils import compile_bir_kernel

    key = (n_loc, m)
    if key not in _NC_CACHE:
        last = None
        for _attempt in range(12):
            nc = build_module(n_loc, m)
            try:
                with tempfile.TemporaryDirectory() as td:
                    compile_bir_kernel(nc.to_json_bytes(), td)
                _NC_CACHE[key] = nc
                break
            except Exception as e:  # noqa: BLE001 — retry on compile flake
                last = e
        else:
            raise RuntimeError(f"no schedule compiled after 12 tries: {last}")
    return _NC_CACHE[key]


def run_hw(query_points, ref_points, ref_flow, trace=False):
    in_maps = pack_inputs(query_points, ref_points, ref_flow)
    n = np.asarray(query_points).shape[0]
    m = np.asarray(ref_points).shape[0]
    nc = _get_module(n // N_CORES, m)
    res = run_bass_kernel_spmd(
        nc, in_maps, core_ids=list(range(N_CORES)), trace=trace
    )
    out = np.concatenate([r["out"] for r in res.results], axis=0)
    return out, res


def kernel(query_points, ref_points, ref_flow, power, k):
    assert int(power) == 2 and int(k) == K
    out, _ = run_hw(query_points, ref_points, ref_flow, trace=False)
    return out

